# revision 3
# baseline (speedup 1.0000x reference)
"""Multi-head attention (B=2, S=2048, D=1024, H=16) on 8 Trainium2 cores.

Sharding: core c -> (batch b = c//4, head-group g = c%4, 4 heads each).
Tensor-parallel over heads within a batch; the output projection is done
per head-group against the matching Wo column slice and the partial
[S, D] results are summed on the host (plus the folded biases bo + Wo@bv).

All on-device matmuls run in float32r (full-rate PE streaming); exp runs on
the scalar engine; the softmax denominator comes from a ones-column appended
to V in the PV matmul.
"""

from contextlib import ExitStack

import numpy as np

import concourse.bacc as bacc
import concourse.tile as tile
from concourse import mybir

D_MODEL = 1024
NUM_HEADS = 16
D_K = 64
B = 2
S_FULL = 2048
N_CORES = 8
GH = 4              # heads per core
GJ = GH * D_K       # 256 columns per head-group

F32 = mybir.dt.float32
F32R = mybir.dt.float32r
AF = mybir.ActivationFunctionType


def build_nc(S=S_FULL, SB=512):
    """Build + compile the per-core program (identical on all 8 cores)."""
    assert S % SB == 0 and SB % 128 == 0
    NB = S // SB      # sq blocks
    ST = S // 128     # sk tiles
    DT = D_MODEL // 128
    JT = GJ // 128    # 2 j-tiles (2 heads each)

    nc = bacc.Bacc("TRN2", target_bir_lowering=False, debug=False)

    xqT = nc.dram_tensor("xqT", [D_MODEL, S], F32R, kind="ExternalInput").ap()
    xkT = nc.dram_tensor("xkT", [D_MODEL, S], F32R, kind="ExternalInput").ap()
    xvT = nc.dram_tensor("xvT", [D_MODEL, S], F32R, kind="ExternalInput").ap()
    wqT = nc.dram_tensor("wqT", [D_MODEL, GJ], F32R, kind="ExternalInput").ap()
    wkT = nc.dram_tensor("wkT", [D_MODEL, GJ], F32R, kind="ExternalInput").ap()
    wvT = nc.dram_tensor("wvT", [D_MODEL, GJ], F32R, kind="ExternalInput").ap()
    woT = nc.dram_tensor("woT", [GJ, D_MODEL], F32R, kind="ExternalInput").ap()
    bq = nc.dram_tensor("bq", [GJ], F32, kind="ExternalInput").ap()
    bk = nc.dram_tensor("bk", [GJ], F32, kind="ExternalInput").ap()
    yT = nc.dram_tensor("yT", [D_MODEL, S], F32, kind="ExternalOutput").ap()

    with tile.TileContext(nc) as tc:
        with ExitStack() as ctx:
            cpool = ctx.enter_context(tc.tile_pool(name="const", bufs=1))
            xs_pool = ctx.enter_context(tc.tile_pool(name="xs", bufs=3))
            p_pool = ctx.enter_context(tc.tile_pool(name="pt", bufs=3))
            y_pool = ctx.enter_context(tc.tile_pool(name="ys", bufs=3))
            s_pool = ctx.enter_context(tc.tile_pool(name="sm", bufs=3))
            ps_a = ctx.enter_context(tc.tile_pool(name="psa", bufs=2, space="PSUM"))
            ps_v = ctx.enter_context(tc.tile_pool(name="psv", bufs=2, space="PSUM"))
            ps_s = ctx.enter_context(tc.tile_pool(name="pss", bufs=2, space="PSUM"))
            ps_o = ctx.enter_context(tc.tile_pool(name="pso", bufs=2, space="PSUM"))

            # ---- persistent SBUF ----
            wq_sb = cpool.tile([128, DT, GJ], F32R, tag="wq")
            wk_sb = cpool.tile([128, DT, GJ], F32R, tag="wk")
            wv_sb = cpool.tile([128, DT, GJ], F32R, tag="wv")
            wo_sb = cpool.tile([128, JT, D_MODEL], F32R, tag="wo")
            bq_sb = cpool.tile([128, JT], F32, tag="bq")
            bk_sb = cpool.tile([128, JT], F32, tag="bk")
            nc.sync.dma_start(wq_sb[:], wqT.rearrange("(t p) j -> p t j", p=128))
            nc.sync.dma_start(wk_sb[:], wkT.rearrange("(t p) j -> p t j", p=128))
            nc.sync.dma_start(wv_sb[:], wvT.rearrange("(t p) j -> p t j", p=128))
            nc.sync.dma_start(wo_sb[:], woT.rearrange("(t p) m -> p t m", p=128))
            nc.sync.dma_start(bq_sb[:], bq.rearrange("(t p) -> p t", p=128))
            nc.sync.dma_start(bk_sb[:], bk.rearrange("(t p) -> p t", p=128))

            qhT_sb = cpool.tile([128, JT, S], F32R, tag="qhT")
            khT_sb = cpool.tile([128, JT, S], F32R, tag="khT")
            vh_sb = cpool.tile([128, ST, GH, 65], F32R, tag="vh")
            oall_sb = cpool.tile([128, JT, S], F32R, tag="oall")

            ones_sb = cpool.tile([128, 1], F32, tag="ones")
            nc.vector.memset(ones_sb[:], 1.0)
            nc.vector.tensor_copy(
                vh_sb[:, :, :, 64:65],
                ones_sb[:, None, :].broadcast_to([128, ST, GH, 1]),
            )

            # ---- stage A: projections (q,k transposed-layout; v normal) ----
            for xT, w_sb, b_sb, outT in (
                (xkT, wk_sb, bk_sb, khT_sb),
                (xvT, wv_sb, None, None),
                (xqT, wq_sb, bq_sb, qhT_sb),
            ):
                for sb in range(NB):
                    ss = slice(sb * SB, (sb + 1) * SB)
                    xs = xs_pool.tile([128, DT, SB], F32R, tag="xs")
                    nc.sync.dma_start(
                        xs[:], xT[:, ss].rearrange("(t p) s -> p t s", p=128)
                    )
                    if outT is not None:
                        # qhT / khT: [GJ, S] transposed projections + bias
                        for jt in range(JT):
                            pa = ps_a.tile([128, SB], F32, tag="pa")
                            for d in range(DT):
                                nc.tensor.matmul(
                                    pa[:],
                                    w_sb[:, d, jt * 128:(jt + 1) * 128],
                                    xs[:, d, :],
                                    start=(d == 0),
                                    stop=(d == DT - 1),
                                )
                            nc.scalar.activation(
                                outT[:, jt, ss], pa[:], AF.Identity,
                                bias=b_sb[:, jt:jt + 1], scale=1.0,
                            )
                    else:
                        # vh: normal layout [S, GJ], x tiles stationary
                        for stl in range(SB // 128):
                            st = sb * (SB // 128) + stl
                            pv = ps_v.tile([128, GJ], F32, tag="pv")
                            for d in range(DT):
                                nc.tensor.matmul(
                                    pv[:],
                                    xs[:, d, stl * 128:(stl + 1) * 128],
                                    wv_sb[:, d, :],
                                    start=(d == 0),
                                    stop=(d == DT - 1),
                                )
                            nc.vector.tensor_copy(
                                vh_sb[:, st, :, 0:64],
                                pv[:].rearrange("p (h e) -> p h e", h=GH),
                            )

            # ---- stage B: attention per (sq-block, head) + stage C per block ----
            for sb in range(NB):
                ss = slice(sb * SB, (sb + 1) * SB)
                for h in range(GH):
                    jt, base = h // 2, 64 * (h % 2)
                    po = ps_o.tile([65, SB], F32, tag="po")
                    for st in range(ST):
                        pss = ps_s.tile([128, SB], F32, tag="ps")
                        nc.tensor.matmul(
                            pss[:],
                            khT_sb[base:base + 64, jt,
                                   st * 128:(st + 1) * 128],
                            qhT_sb[base:base + 64, jt, ss],
                            start=True, stop=True,
                        )
                        pt = p_pool.tile([128, SB], F32R, tag="pt")
                        nc.scalar.activation(pt[:], pss[:], AF.Exp, scale=0.125)
                        nc.tensor.matmul(
                            po[:],
                            vh_sb[:, st, h, :],
                            pt[:],
                            start=(st == 0),
                            stop=(st == ST - 1),
                        )
                    rcp = s_pool.tile([1, SB], F32, tag="rcp")
                    nc.vector.reciprocal(rcp[:], po[64:65, :])
                    bcast = s_pool.tile([64, SB], F32, tag="bcast")
                    nc.gpsimd.partition_broadcast(bcast[:], rcp[:])
                    nc.vector.tensor_mul(
                        oall_sb[base:base + 64, jt, ss], po[0:64, :], bcast[:]
                    )
                # stage C: y_T[:, block] = woT.T @ o_norm_T[:, block]
                for mt in range(DT):
                    pa = ps_a.tile([128, SB], F32, tag="pa")
                    for kt in range(JT):
                        nc.tensor.matmul(
                            pa[:],
                            wo_sb[:, kt, mt * 128:(mt + 1) * 128],
                            oall_sb[:, kt, ss],
                            start=(kt == 0),
                            stop=(kt == JT - 1),
                        )
                    yt = y_pool.tile([128, SB], F32, tag="yt")
                    nc.vector.tensor_copy(yt[:], pa[:])
                    nc.sync.dma_start(
                        yT.rearrange("(t p) s -> t p s", p=128)[mt, :, ss], yt[:]
                    )

    nc.compile()
    return nc


_NC_CACHE = {}


def _get_nc(S=S_FULL):
    if S not in _NC_CACHE:
        _NC_CACHE[S] = build_nc(S)
    return _NC_CACHE[S]


def make_in_maps(q, k, v, Wq, bq, Wk, bk, Wv, bv, Wo, bo, S=S_FULL):
    q = np.asarray(q, np.float32)
    k = np.asarray(k, np.float32)
    v = np.asarray(v, np.float32)
    Wq = np.asarray(Wq, np.float32)
    Wk = np.asarray(Wk, np.float32)
    Wv = np.asarray(Wv, np.float32)
    Wo = np.asarray(Wo, np.float32)
    bq = np.asarray(bq, np.float32)
    bk = np.asarray(bk, np.float32)

    in_maps = []
    for c in range(N_CORES):
        b, g = divmod(c, GH)
        sl = slice(g * GJ, (g + 1) * GJ)
        in_maps.append({
            "xqT": np.ascontiguousarray(q[b, :S].T),
            "xkT": np.ascontiguousarray(k[b, :S].T),
            "xvT": np.ascontiguousarray(v[b, :S].T),
            "wqT": np.ascontiguousarray(Wq[sl].T),
            "wkT": np.ascontiguousarray(Wk[sl].T),
            "wvT": np.ascontiguousarray(Wv[sl].T),
            "woT": np.ascontiguousarray(Wo[:, sl].T),
            "bq": np.ascontiguousarray(bq[sl]),
            "bk": np.ascontiguousarray(bk[sl]),
        })
    return in_maps


def gather_out(results, Wo, bv, bo, S=S_FULL):
    Wo = np.asarray(Wo, np.float32)
    bv = np.asarray(bv, np.float32)
    bo = np.asarray(bo, np.float32)
    out = np.zeros((B, S, D_MODEL), np.float32)
    for c in range(N_CORES):
        out[c // GH] += results[c]["yT"].T
    out += bo + Wo @ bv
    return out


def kernel(q, k, v, Wq, bq, Wk, bk, Wv, bv, Wo, bo):
    from concourse.bass_utils import run_bass_kernel_spmd

    nc = _get_nc(S_FULL)
    in_maps = make_in_maps(q, k, v, Wq, bq, Wk, bk, Wv, bv, Wo, bo)
    res = run_bass_kernel_spmd(nc, in_maps, core_ids=list(range(N_CORES)))
    return gather_out(res.results, Wo, bv, bo)
